# revision 1
# baseline (speedup 1.0000x reference)
"""MultiHeadDecoder (moe_routing) Trainium2 kernel.

Strategy: expert-parallel. Each of the 8 cores owns one head's weights.
Host groups samples by head index, pads each group to a common capacity C
(multiple of 64), and transposes X so the contraction dim lands on
partitions. Each core runs a dense 2-layer MLP (256->512 relu, 512->2048)
for its head's samples. Host scatters rows back to original order.

Layer 1 computes H^T (hid on partitions) so layer 2 can contract over hid
without an on-chip transpose:
  H^T[hc]  = W1[:, hc].T @ X^T      (lhsT=W1 chunk, rhs=X^T chunk)
  out[st]  = (H^T[:, st]).T @ W2    (lhsT=H^T chunk, rhs=W2 chunk)

Matmuls run in float32r (fp32 bits, full PE rate, tf32-ish multiply).
Inputs are packed host-side into the exact SBUF layout so every DMA has
long contiguous runs per partition. All inputs stream on the sync (SP)
HWDGE ring; all output stores go on the scalar (Act) ring so they never
queue behind the W2 stream (rings are FIFO per issuing engine). Stage B
is ordered oc-outer so only the first W2 chunk's DMA gates its start.
Dummy matmuls keep the PE's HAM clock-gate warm while DMAs stream.
"""

import numpy as np

import concourse.bass as bass
import concourse.mybir as mybir
from concourse import bacc
from concourse.tile import TileContext
from concourse.bass_utils import run_bass_kernel_spmd

IN_F, HID, OUT_F, N_HEADS, BATCH = 256, 512, 2048, 8, 4096
N_CORES = 8
P = 128
KI = IN_F // P     # 2  input-feature chunks
HC = HID // P      # 4  hidden chunks
OC = OUT_F // 512  # 4  output-feature chunks of 512

f32 = mybir.dt.float32
f32r = mybir.dt.float32r  # fp32 bits, PE runs at full (bf16) rate, tf32-ish mul

_NC_CACHE: dict = {}


def build_nc(C: int):
    """Build the per-core Bass program for capacity C (multiple of 64)."""
    KF = C + HID     # free size of one k-part: xt_k then w1_k
    stiles = [(s, min(P, C - s)) for s in range(0, C, P)]
    sgroups = [(s, min(512, C - s)) for s in range(0, C, 512)]

    nc = bacc.Bacc("TRN2", target_bir_lowering=False, debug=False,
                   num_devices=N_CORES)
    xin = nc.dram_tensor("xin", [KI, P, KF], f32r, kind="ExternalInput")
    b1s = nc.dram_tensor("b1s", [P, HC], f32, kind="ExternalInput")
    w2p = nc.dram_tensor("w2p", [OC, P, HC * 512], f32r, kind="ExternalInput")
    b2 = nc.dram_tensor("b2", [1, OUT_F], f32, kind="ExternalInput")
    out = nc.dram_tensor("out", [C, OUT_F], f32, kind="ExternalOutput")

    relu = mybir.ActivationFunctionType.Relu

    with TileContext(nc) as tc:
        with (
            tc.tile_pool(name="const", bufs=1) as const,
            tc.tile_pool(name="psumA", bufs=2, space="PSUM") as psumA,
            tc.tile_pool(name="psumB", bufs=5, space="PSUM") as psumB,
            tc.tile_pool(name="psumW", bufs=1, space="PSUM") as psumW,
            tc.tile_pool(name="outp", bufs=6) as outp,
        ):
            # HAM warmup: dummy matmuls with no DMA deps keep the PE busy
            # while inputs stream in, so real matmuls run at 2.4 GHz.
            wsrc = const.tile([P, 64], f32, tag="warm")
            nc.vector.memset(wsrc[:], 0.0)
            wps = psumW.tile([64, 64], f32, tag="warmps")
            for _ in range(35):
                nc.tensor.matmul(wps[:], lhsT=wsrc[:, :64], rhs=wsrc[:],
                                 start=True, stop=True)

            # Stage-A inputs first so the PE starts ASAP; W2 streams behind.
            # Two k-part DMAs so the k=0 matmuls can start at half-arrival.
            xin_ks = []
            for k in range(KI):
                xk = const.tile([P, KF], f32r, tag=f"xin_{k}")
                nc.sync.dma_start(xk[:], xin[k])
                xin_ks.append(xk)
            b1_s = const.tile([P, HC], f32)
            nc.sync.dma_start(b1_s[:], b1s[:])
            b2_row = const.tile([1, OUT_F], f32)
            nc.sync.dma_start(b2_row[:], b2[:])
            b2_s = const.tile([P, OUT_F], f32)
            nc.gpsimd.partition_broadcast(b2_s[:], b2_row[:])
            w2_cs = []
            for oc in range(OC):
                w2_c = const.tile([P, HC * 512], f32r, tag=f"w2_{oc}")
                nc.sync.dma_start(w2_c[:], w2p[oc])
                w2_cs.append(w2_c)

            # Stage A: H^T [hid(part), sample(free)], relu(x @ W1 + b1)
            # sgroups outer so stage B's early sample tiles are ready sooner.
            ht = const.tile([P, HC, C], f32r)
            for (s0, sn) in sgroups:
                for hc in range(HC):
                    ps = psumA.tile([P, 512], f32, tag="psA")
                    for k in range(KI):
                        nc.tensor.matmul(
                            ps[:, :sn],
                            lhsT=xin_ks[k][:, C + hc * P: C + (hc + 1) * P],
                            rhs=xin_ks[k][:, s0:s0 + sn],
                            start=(k == 0), stop=(k == KI - 1),
                        )
                    nc.scalar.activation(
                        ht[:, hc, s0:s0 + sn], ps[:, :sn], relu,
                        bias=b1_s[:, hc:hc + 1],
                    )

            # Bridge warmup: keep the PE hot while the first W2 chunk lands.
            for _ in range(12):
                nc.tensor.matmul(wps[:], lhsT=wsrc[:, :64], rhs=wsrc[:],
                                 start=True, stop=True)

            # Stage B: out[st, oc] = H[st] @ W2[:, oc] + b2[oc]
            for oc in range(OC):
                for (s0, sn) in stiles:
                    ps = psumB.tile([P, 512], f32, tag="psB")
                    for hc in range(HC):
                        nc.tensor.matmul(
                            ps[:sn, :],
                            lhsT=ht[:, hc, s0:s0 + sn],
                            rhs=w2_cs[oc][:, hc * 512:(hc + 1) * 512],
                            start=(hc == 0), stop=(hc == HC - 1),
                        )
                    ot = outp.tile([P, 512], f32, tag="ot")
                    nc.vector.tensor_add(
                        out=ot[:sn, :],
                        in0=ps[:sn, :],
                        in1=b2_s[:sn, oc * 512:(oc + 1) * 512],
                    )
                    nc.scalar.dma_start(
                        out[s0:s0 + sn, oc * 512:(oc + 1) * 512], ot[:sn, :]
                    )

    nc.compile()
    return nc


def kernel(X, X_head_idx, W1, b1, W2, b2):
    X = np.ascontiguousarray(np.asarray(X, dtype=np.float32))
    idx = np.asarray(X_head_idx).astype(np.int64)
    W1 = np.asarray(W1, dtype=np.float32)
    b1 = np.asarray(b1, dtype=np.float32)
    W2 = np.asarray(W2, dtype=np.float32)
    b2 = np.asarray(b2, dtype=np.float32)

    batch = X.shape[0]
    counts = np.bincount(idx, minlength=N_HEADS)
    order = np.argsort(idx, kind="stable")
    positions = np.split(order, np.cumsum(counts)[:-1])

    C = max(512, int(-(-counts.max() // 64)) * 64)
    if C not in _NC_CACHE:
        _NC_CACHE[C] = build_nc(C)
    nc = _NC_CACHE[C]

    in_maps = []
    for h in range(N_HEADS):
        pos = positions[h]
        # xin[k, p, :] = [ X[pos, k*128+p] (len C, padded) | W1[h, k*128+p, :] ]
        xin = np.zeros((KI, P, C + HID), dtype=np.float32)
        if len(pos):
            xk = X[pos].T.reshape(KI, P, len(pos))          # [k, p, c]
            xin[:, :, :len(pos)] = xk
        xin[:, :, C:] = W1[h].reshape(KI, P, HID)
        # w2 packed: [oc, p, hc*512 + o'] = W2[h, hc*128 + p, oc*512 + o']
        w2t = np.transpose(W2[h].reshape(HC, P, OUT_F), (1, 0, 2))  # [p, hc, of]
        w2p = np.empty((OC, P, HC * 512), dtype=np.float32)
        for oc in range(OC):
            w2p[oc] = w2t[:, :, oc * 512:(oc + 1) * 512].reshape(P, HC * 512)
        in_maps.append({
            "xin": xin,
            "b1s": np.ascontiguousarray(b1[h].reshape(HC, P).T),
            "w2p": w2p,
            "b2": np.ascontiguousarray(b2[h][None, :]),
        })

    try:
        res = run_bass_kernel_spmd(nc, in_maps, list(range(N_CORES)))
    except Exception:
        res = run_bass_kernel_spmd(nc, in_maps, list(range(N_CORES)))

    out = np.empty((batch, OUT_F), dtype=np.float32)
    for h in range(N_HEADS):
        pos = positions[h]
        if len(pos):
            out[pos] = res.results[h]["out"][:len(pos)]
    return out



# revision 8
# speedup vs baseline: 1.1483x; 1.1483x over previous
"""MultiHeadDecoder (moe_routing) Trainium2 kernel.

Strategy: expert-parallel. Each of the 8 cores owns one head's weights.
Host groups samples by head index, pads each group to a common capacity C
(multiple of 16, >= 512), and transposes X so the contraction dim lands on
partitions. Each core runs a dense 2-layer MLP (256->512 relu, 512->2048)
for its head's samples. Host scatters rows back to original order and adds
b2 on device via a DVE tensor_add (which doubles as the PSUM->SBUF move).

Layer 1 computes H^T (hid on partitions) so layer 2 can contract over hid
without an on-chip transpose:
  H^T[hc]  = W1[:, hc].T @ X^T      (lhsT=W1 chunk, rhs=X^T chunk)
  out[st]  = (H^T[:, st]).T @ W2    (lhsT=H^T chunk, rhs=W2 chunk)

All matmul operands are float16: full PE rate at any free size (f32r
degrades 4x below 256 columns), half the HBM traffic of f32, and lower PE
power (less HAM duty-cycle throttling). PSUM accumulates in f32.

Startup choreography (the baseline lost ~7us here):
 - bf16 dummy matmuls (single-pass, unlike fp32 dummies which split
   LOW/HIGH) keep the PE busy for the HAM ramp while inputs stream, sized
   to end about when xin lands rather than blocking stage A.
 - A dummy activation right after the warm memset pulls ACT_TABLE_LOAD
   (~1.3us) off the critical path of the first real relu.
 - Only SP (sync) and Activation (scalar) have HWDGE rings. Inputs are
   split across both so the two xin k-parts land simultaneously: sync
   carries xin k0 + W2 c0/c2/c3, scalar carries xin k1 + b1s + b2bc +
   W2 c1. Output stores alternate between the two rings.
"""

import ml_dtypes
import numpy as np

import concourse.bass as bass
import concourse.mybir as mybir
from concourse import bacc
from concourse.tile import TileContext
from concourse.bass_utils import run_bass_kernel_spmd

IN_F, HID, OUT_F, N_HEADS, BATCH = 256, 512, 2048, 8, 4096
N_CORES = 8
P = 128
KI = IN_F // P     # 2  input-feature chunks
HC = HID // P      # 4  hidden chunks
OC = OUT_F // 512  # 4  output-feature chunks of 512

f32 = mybir.dt.float32
f16 = mybir.dt.float16
bf16 = mybir.dt.bfloat16

N_WARM = 30

_NC_CACHE: dict = {}


def build_nc(C: int):
    """Build the per-core Bass program for capacity C (mult of 16, >=512)."""
    KF = C + HID     # free size of one k-part: xt_k then w1_k
    stiles = [(s, min(P, C - s)) for s in range(0, C, P)]
    sgroups = [(s, min(512, C - s)) for s in range(0, C, 512)]

    nc = bacc.Bacc("TRN2", target_bir_lowering=False, debug=False,
                   num_devices=N_CORES)
    xin = nc.dram_tensor("xin", [KI, P, KF], f16, kind="ExternalInput")
    b1s = nc.dram_tensor("b1s", [P, HC], f32, kind="ExternalInput")
    w2p = nc.dram_tensor("w2p", [OC, P, HC * 512], f16, kind="ExternalInput")
    b2bc = nc.dram_tensor("b2bc", [P, OUT_F], bf16, kind="ExternalInput")
    out = nc.dram_tensor("out", [C, OUT_F], f32, kind="ExternalOutput")

    relu = mybir.ActivationFunctionType.Relu

    with TileContext(nc) as tc:
        with (
            tc.tile_pool(name="const", bufs=1) as const,
            tc.tile_pool(name="psumA", bufs=3, space="PSUM") as psumA,
            tc.tile_pool(name="psumB", bufs=4, space="PSUM") as psumB,
            tc.tile_pool(name="psumW", bufs=1, space="PSUM") as psumW,
            tc.tile_pool(name="outp", bufs=6) as outp,
        ):
            # Warm tile on gpsimd (otherwise idle) so the sync/scalar rings
            # can start their DMA triggers immediately.
            wsrc = const.tile([P, 64], bf16, tag="warm")
            nc.gpsimd.memset(wsrc[:], 0.0)

            # Sync ring: xin k0, then W2 c0/c2/c3.
            xin_ks = [const.tile([P, KF], f16, tag=f"xin_{k}",
                                 name=f"xin_{k}") for k in range(KI)]
            w2_cs = [const.tile([P, HC * 512], f16, tag=f"w2_{oc}",
                                name=f"w2_{oc}") for oc in range(OC)]
            nc.sync.dma_start(xin_ks[0][:], xin[0])
            nc.sync.dma_start(w2_cs[0][:], w2p[0])
            nc.sync.dma_start(w2_cs[2][:], w2p[2])
            nc.sync.dma_start(w2_cs[3][:], w2p[3])

            # Scalar ring: xin k1, b1s, act-table preload, b2bc, W2 c1.
            nc.scalar.dma_start(xin_ks[1][:], xin[1])
            b1_s = const.tile([P, HC], f32)
            nc.scalar.dma_start(b1_s[:], b1s[:])
            wact = const.tile([P, 8], f32, tag="wact")
            nc.scalar.activation(wact[:], wsrc[:, :8], relu)
            b2_s = const.tile([P, OUT_F], bf16)
            nc.scalar.dma_start(b2_s[:], b2bc[:])
            nc.scalar.dma_start(w2_cs[1][:], w2p[1])

            # HAM warmup: bf16 single-pass dummies with no DMA deps keep
            # the PE busy (clock ramp) while xin streams in.
            wps = psumW.tile([64, 64], f32, tag="warmps")
            for _ in range(N_WARM):
                nc.tensor.matmul(wps[:], lhsT=wsrc[:, :64], rhs=wsrc[:, :64],
                                 start=True, stop=True)

            # Stage A: H^T [hid(part), sample(free)], relu(x @ W1 + b1).
            ht = const.tile([P, HC, C], f16)
            for (s0, sn) in sgroups:
                for hc in range(HC):
                    ps = psumA.tile([P, 512], f32, tag="psA")
                    for k in range(KI):
                        nc.tensor.matmul(
                            ps[:, :sn],
                            lhsT=xin_ks[k][:, C + hc * P: C + (hc + 1) * P],
                            rhs=xin_ks[k][:, s0:s0 + sn],
                            start=(k == 0), stop=(k == KI - 1),
                        )
                    nc.scalar.activation(
                        ht[:, hc, s0:s0 + sn], ps[:, :sn], relu,
                        bias=b1_s[:, hc:hc + 1],
                    )

            # Stage B: out[st, oc] = H[st] @ W2[:, oc] + b2[oc]
            # oc-outer so only W2 chunk oc gates each quarter; stiles in
            # order with the small tail last (smallest final transfer).
            ndma = 0
            for oc in range(OC):
                for (s0, sn) in stiles:
                    ps = psumB.tile([P, 512], f32, tag="psB")
                    for hc in range(HC):
                        nc.tensor.matmul(
                            ps[:sn, :],
                            lhsT=ht[:, hc, s0:s0 + sn],
                            rhs=w2_cs[oc][:, hc * 512:(hc + 1) * 512],
                            start=(hc == 0), stop=(hc == HC - 1),
                        )
                    ot = outp.tile([P, 512], f32, tag="ot")
                    nc.vector.tensor_add(
                        out=ot[:sn, :],
                        in0=ps[:sn, :],
                        in1=b2_s[:sn, oc * 512:(oc + 1) * 512],
                    )
                    eng = nc.scalar if ndma % 2 == 0 else nc.sync
                    eng.dma_start(
                        out[s0:s0 + sn, oc * 512:(oc + 1) * 512], ot[:sn, :]
                    )
                    ndma += 1

    nc.compile()
    return nc


def kernel(X, X_head_idx, W1, b1, W2, b2):
    X = np.ascontiguousarray(np.asarray(X, dtype=np.float32))
    idx = np.asarray(X_head_idx).astype(np.int64)
    W1 = np.asarray(W1, dtype=np.float32)
    b1 = np.asarray(b1, dtype=np.float32)
    W2 = np.asarray(W2, dtype=np.float32)
    b2 = np.asarray(b2, dtype=np.float32)

    batch = X.shape[0]
    counts = np.bincount(idx, minlength=N_HEADS)
    order = np.argsort(idx, kind="stable")
    positions = np.split(order, np.cumsum(counts)[:-1])

    C = max(512, int(-(-counts.max() // 16)) * 16)
    if C not in _NC_CACHE:
        _NC_CACHE[C] = build_nc(C)
    nc = _NC_CACHE[C]

    in_maps = []
    for h in range(N_HEADS):
        pos = positions[h]
        # xin[k, p, :] = [ X[pos, k*128+p] (len C, padded) | W1[h, k*128+p, :] ]
        xin = np.zeros((KI, P, C + HID), dtype=np.float16)
        if len(pos):
            xk = X[pos].T.reshape(KI, P, len(pos))          # [k, p, c]
            xin[:, :, :len(pos)] = xk
        xin[:, :, C:] = W1[h].reshape(KI, P, HID)
        # w2 packed: [oc, p, hc*512 + o'] = W2[h, hc*128 + p, oc*512 + o']
        w2t = np.transpose(W2[h].reshape(HC, P, OUT_F), (1, 0, 2))  # [p, hc, of]
        w2p = np.empty((OC, P, HC * 512), dtype=np.float16)
        for oc in range(OC):
            w2p[oc] = w2t[:, :, oc * 512:(oc + 1) * 512].reshape(P, HC * 512)
        in_maps.append({
            "xin": xin,
            "b1s": np.ascontiguousarray(b1[h].reshape(HC, P).T),
            "w2p": w2p,
            "b2bc": np.ascontiguousarray(np.broadcast_to(
                b2[h][None, :].astype(ml_dtypes.bfloat16), (P, OUT_F))),
        })

    try:
        res = run_bass_kernel_spmd(nc, in_maps, list(range(N_CORES)))
    except Exception:
        res = run_bass_kernel_spmd(nc, in_maps, list(range(N_CORES)))

    out = np.empty((batch, OUT_F), dtype=np.float32)
    for h in range(N_HEADS):
        pos = positions[h]
        if len(pos):
            out[pos] = res.results[h]["out"][:len(pos)]
    return out


# revision 12
# speedup vs baseline: 1.1674x; 1.0166x over previous
"""MultiHeadDecoder (moe_routing) Trainium2 kernel.

Strategy: expert-parallel. Each of the 8 cores owns one head's weights.
Host groups samples by head index, pads each group to a common capacity C
(multiple of 16, >= 512), and transposes X so the contraction dim lands on
partitions. Each core runs a dense 2-layer MLP (256->512 relu, 512->2048)
for its head's samples. Host scatters rows back to original order and adds
b2 on device via a DVE tensor_add (which doubles as the PSUM->SBUF move).

Layer 1 computes H^T (hid on partitions) so layer 2 can contract over hid
without an on-chip transpose:
  H^T[hc]  = W1[:, hc].T @ X^T      (lhsT=W1 chunk, rhs=X^T chunk)
  out[st]  = (H^T[:, st]).T @ W2    (lhsT=H^T chunk, rhs=W2 chunk)

All matmul operands are float16: full PE rate at any free size (f32r
degrades 4x below 256 columns), half the HBM traffic of f32, and lower PE
power (less HAM duty-cycle throttling). PSUM accumulates in f32.

Startup choreography (the baseline lost ~7us here):
 - bf16 dummy matmuls (single-pass, unlike fp32 dummies which split
   LOW/HIGH) keep the PE busy for the HAM ramp while inputs stream, sized
   to end about when xin lands rather than blocking stage A.
 - A dummy activation right after the warm memset pulls ACT_TABLE_LOAD
   (~1.3us) off the critical path of the first real relu.
 - Only SP (sync) and Activation (scalar) have HWDGE rings. Inputs are
   split across both so the two xin k-parts land simultaneously: sync
   carries xin k0 + W2 c0/c2/c3, scalar carries xin k1 + b1s + b2bc +
   W2 c1. Output stores alternate between the two rings.
"""

import ml_dtypes
import numpy as np

import concourse.bass as bass
import concourse.mybir as mybir
from concourse import bacc
from concourse.tile import TileContext
from concourse.bass_utils import run_bass_kernel_spmd

IN_F, HID, OUT_F, N_HEADS, BATCH = 256, 512, 2048, 8, 4096
N_CORES = 8
P = 128
KI = IN_F // P     # 2  input-feature chunks
HC = HID // P      # 4  hidden chunks
OC = OUT_F // 512  # 4  output-feature chunks of 512

f32 = mybir.dt.float32
f16 = mybir.dt.float16
bf16 = mybir.dt.bfloat16

N_WARM0 = 12   # tiny matmuls on the framework's const AP (no memset dep)
N_WARM = 62    # 64-col dummies bridging until xin lands (~10.2us)

_NC_CACHE: dict = {}


def build_nc(C: int):
    """Build the per-core Bass program for capacity C (mult of 16, >=512)."""
    KF = C + HID     # free size of one k-part: xt_k then w1_k
    stiles = [(s, min(P, C - s)) for s in range(0, C, P)]
    sgroups = [(s, min(512, C - s)) for s in range(0, C, 512)]

    nc = bacc.Bacc("TRN2", target_bir_lowering=False, debug=False,
                   num_devices=N_CORES)
    xin = nc.dram_tensor("xin", [KI, P, KF], f16, kind="ExternalInput")
    b1s = nc.dram_tensor("b1s", [P, HC], f32, kind="ExternalInput")
    w2p = nc.dram_tensor("w2p", [OC, P, HC * 512], f16, kind="ExternalInput")
    b2bc = nc.dram_tensor("b2bc", [P, OUT_F], bf16, kind="ExternalInput")
    out = nc.dram_tensor("out", [C, OUT_F], f32, kind="ExternalOutput")

    relu = mybir.ActivationFunctionType.Relu

    with TileContext(nc) as tc:
        with (
            tc.tile_pool(name="const", bufs=1) as const,
            tc.tile_pool(name="psumA", bufs=3, space="PSUM") as psumA,
            tc.tile_pool(name="psumB", bufs=4, space="PSUM") as psumB,
            tc.tile_pool(name="psumW", bufs=1, space="PSUM") as psumW,
            tc.tile_pool(name="outp", bufs=6) as outp,
        ):
            # Warm tile on gpsimd (otherwise idle) so the sync/scalar rings
            # can start their DMA triggers immediately.
            wsrc = const.tile([P, 64], bf16, tag="warm")
            nc.gpsimd.memset(wsrc[:], 0.0)

            # Sync ring: xin k0, then W2 c0/c2/c3.
            xin_ks = [const.tile([P, KF], f16, tag=f"xin_{k}",
                                 name=f"xin_{k}") for k in range(KI)]
            w2_cs = [const.tile([P, HC * 512], f16, tag=f"w2_{oc}",
                                name=f"w2_{oc}") for oc in range(OC)]
            nc.sync.dma_start(xin_ks[0][:], xin[0])
            nc.sync.dma_start(w2_cs[0][:], w2p[0])
            nc.sync.dma_start(w2_cs[2][:], w2p[2])
            nc.sync.dma_start(w2_cs[3][:], w2p[3])

            # Scalar ring: xin k1, b1s, act-table preload, b2bc, W2 c1.
            nc.scalar.dma_start(xin_ks[1][:], xin[1])
            b1_s = const.tile([P, HC], f32)
            nc.scalar.dma_start(b1_s[:], b1s[:])
            wact = const.tile([P, 8], f32, tag="wact")
            nc.scalar.activation(wact[:], wsrc[:, :8], relu)
            b2_s = const.tile([P, OUT_F], bf16)
            nc.scalar.dma_start(b2_s[:], b2bc[:])
            nc.scalar.dma_start(w2_cs[1][:], w2p[1])

            # HAM warmup: the PE is promoted to full clock only after ~6us
            # of gap-free activity, so keep it busy from the earliest
            # possible moment until xin lands. Phase 1 uses the framework's
            # preamble-memset const AP (no dependency on our own memset);
            # phase 2 uses the bf16 warm tile.
            cone = nc.const_aps.aps[(bf16, 1.0)]
            wps = psumW.tile([64, 64], f32, tag="warmps")
            for _ in range(N_WARM0):
                nc.tensor.matmul(wps[:1, :1], lhsT=cone[:, :1],
                                 rhs=cone[:, :1], start=True, stop=True)
            for _ in range(N_WARM):
                nc.tensor.matmul(wps[:], lhsT=wsrc[:, :64], rhs=wsrc[:, :64],
                                 start=True, stop=True)

            # Stage A: H^T [hid(part), sample(free)], relu(x @ W1 + b1).
            ht = const.tile([P, HC, C], f16)
            for (s0, sn) in sgroups:
                for hc in range(HC):
                    ps = psumA.tile([P, 512], f32, tag="psA")
                    for k in range(KI):
                        nc.tensor.matmul(
                            ps[:, :sn],
                            lhsT=xin_ks[k][:, C + hc * P: C + (hc + 1) * P],
                            rhs=xin_ks[k][:, s0:s0 + sn],
                            start=(k == 0), stop=(k == KI - 1),
                        )
                    nc.scalar.activation(
                        ht[:, hc, s0:s0 + sn], ps[:, :sn], relu,
                        bias=b1_s[:, hc:hc + 1],
                    )

            # Stage B: out[st, oc] = H[st] @ W2[:, oc] + b2[oc]
            # oc-outer so only W2 chunk oc gates each quarter; stiles in
            # order with the small tail last (smallest final transfer).
            ndma = 0
            for oc in range(OC):
                for (s0, sn) in stiles:
                    ps = psumB.tile([P, 512], f32, tag="psB")
                    for hc in range(HC):
                        nc.tensor.matmul(
                            ps[:sn, :],
                            lhsT=ht[:, hc, s0:s0 + sn],
                            rhs=w2_cs[oc][:, hc * 512:(hc + 1) * 512],
                            start=(hc == 0), stop=(hc == HC - 1),
                        )
                    ot = outp.tile([P, 512], f32, tag="ot")
                    last = (oc == OC - 1 and s0 + sn >= C)
                    # Split the final tile's add+store so the tail chain
                    # (DVE add -> DMA trigger -> transfer) is shorter.
                    halves = ((0, 256), (256, 256)) if last else ((0, 512),)
                    for (f0, fn) in halves:
                        nc.vector.tensor_add(
                            out=ot[:sn, f0:f0 + fn],
                            in0=ps[:sn, f0:f0 + fn],
                            in1=b2_s[:sn, oc * 512 + f0: oc * 512 + f0 + fn],
                        )
                        eng = nc.scalar if ndma % 2 == 0 else nc.sync
                        eng.dma_start(
                            out[s0:s0 + sn, oc * 512 + f0: oc * 512 + f0 + fn],
                            ot[:sn, f0:f0 + fn],
                        )
                        ndma += 1

    nc.compile()
    return nc


def kernel(X, X_head_idx, W1, b1, W2, b2):
    X = np.ascontiguousarray(np.asarray(X, dtype=np.float32))
    idx = np.asarray(X_head_idx).astype(np.int64)
    W1 = np.asarray(W1, dtype=np.float32)
    b1 = np.asarray(b1, dtype=np.float32)
    W2 = np.asarray(W2, dtype=np.float32)
    b2 = np.asarray(b2, dtype=np.float32)

    batch = X.shape[0]
    counts = np.bincount(idx, minlength=N_HEADS)
    order = np.argsort(idx, kind="stable")
    positions = np.split(order, np.cumsum(counts)[:-1])

    C = max(512, int(-(-counts.max() // 16)) * 16)
    if C not in _NC_CACHE:
        _NC_CACHE[C] = build_nc(C)
    nc = _NC_CACHE[C]

    in_maps = []
    for h in range(N_HEADS):
        pos = positions[h]
        # xin[k, p, :] = [ X[pos, k*128+p] (len C, padded) | W1[h, k*128+p, :] ]
        xin = np.zeros((KI, P, C + HID), dtype=np.float16)
        if len(pos):
            xk = X[pos].T.reshape(KI, P, len(pos))          # [k, p, c]
            xin[:, :, :len(pos)] = xk
        xin[:, :, C:] = W1[h].reshape(KI, P, HID)
        # w2 packed: [oc, p, hc*512 + o'] = W2[h, hc*128 + p, oc*512 + o']
        w2t = np.transpose(W2[h].reshape(HC, P, OUT_F), (1, 0, 2))  # [p, hc, of]
        w2p = np.empty((OC, P, HC * 512), dtype=np.float16)
        for oc in range(OC):
            w2p[oc] = w2t[:, :, oc * 512:(oc + 1) * 512].reshape(P, HC * 512)
        in_maps.append({
            "xin": xin,
            "b1s": np.ascontiguousarray(b1[h].reshape(HC, P).T),
            "w2p": w2p,
            "b2bc": np.ascontiguousarray(np.broadcast_to(
                b2[h][None, :].astype(ml_dtypes.bfloat16), (P, OUT_F))),
        })

    try:
        res = run_bass_kernel_spmd(nc, in_maps, list(range(N_CORES)))
    except Exception:
        res = run_bass_kernel_spmd(nc, in_maps, list(range(N_CORES)))

    out = np.empty((batch, OUT_F), dtype=np.float32)
    for h in range(N_HEADS):
        pos = positions[h]
        if len(pos):
            out[pos] = res.results[h]["out"][:len(pos)]
    return out


# revision 14
# speedup vs baseline: 1.2282x; 1.0521x over previous
"""MultiHeadDecoder (moe_routing) Trainium2 kernel.

Strategy: expert-parallel. Each of the 8 cores owns one head's weights.
Host groups samples by head index, pads each group to a common capacity C
(multiple of 16, >= 512), and transposes X so the contraction dim lands on
partitions. Each core runs a dense 2-layer MLP (256->512 relu, 512->2048)
for its head's samples. The kernel returns out^T [OUT_F, C]; the host
transposes and scatters rows back to original order.

Layer 1 computes H^T (hid on partitions):
  H^T[hc] = W1[:, hc].T @ X^T          (lhsT=W1 chunk, rhs=X^T chunk)
Layer 2 keeps out-features on partitions and streams samples:
  out^T[of] = W2[:, of].T @ H^T        (lhsT=W2 [hid,of] tile, rhs=H^T)
so the per-matmul cost scales with the true sample count (split into two
~C/2 column groups to fit PSUM banks) instead of paying full 512-column
matmuls for a mostly-empty tail sample tile. The b2 bias is per-partition
in this layout, so a DVE tensor_scalar_add doubles as the PSUM->SBUF move.

All matmul operands are float16: full PE rate at any free size, FWL weight
loads, half the HBM traffic of f32, and lower PE power (less HAM duty
throttling). PSUM accumulates in f32.

Startup choreography (the original baseline lost ~7us here):
 - The PE is promoted to full clock only after ~6us of gap-free activity,
   so dummy matmuls run from the earliest possible moment (phase 1 reads a
   framework const AP that is memset during the preamble) until xin lands.
 - A dummy activation pulls ACT_TABLE_LOAD (~1.3us) off the critical path.
 - b1/b2 are folded into the xin tails: separate tiny-element DMAs have
   brutal per-descriptor cost and gated stage A by ~2us.
 - Inputs stream on both HWDGE rings in parallel (sync: xin k0 + W2
   c0/c2/c3; scalar: xin k1 + W2 c1). Output stores alternate rings, and
   the final of-tile's add+store is split to shorten the tail chain.
"""

import ml_dtypes
import numpy as np

import concourse.bass as bass
import concourse.mybir as mybir
from concourse import bacc
from concourse.tile import TileContext
from concourse.bass_utils import run_bass_kernel_spmd

IN_F, HID, OUT_F, N_HEADS, BATCH = 256, 512, 2048, 8, 4096
N_CORES = 8
P = 128
KI = IN_F // P     # 2  input-feature chunks
HC = HID // P      # 4  hidden chunks
OC = OUT_F // 512  # 4  W2 dma chunks of 512 out-features
OFT = OUT_F // P   # 16 out-feature tiles

f32 = mybir.dt.float32
f16 = mybir.dt.float16
bf16 = mybir.dt.bfloat16

N_WARM0 = 12   # tiny matmuls on the framework's const AP (no memset dep)
N_WARM = 62    # 64-col dummies bridging until xin lands (~11us)

_NC_CACHE: dict = {}


def _sgroups(C: int):
    """Split C sample columns into <=512-wide groups for PSUM banks."""
    if C <= 512:
        return [(0, C)]
    g0 = ((C // 2 + 15) // 16) * 16
    return [(0, g0), (g0, C - g0)]


def build_nc(C: int):
    """Build the per-core Bass program for capacity C (mult of 16, >=512)."""
    # xin[k] free layout: [ X^T (C) | W1 k-part (HID) | bias (16) ]
    # k=0 bias cols: b1 (HC=4 used); k=1 bias cols: b2 per of-tile (16).
    KF = C + HID + 16
    sgroups = _sgroups(C)

    nc = bacc.Bacc("TRN2", target_bir_lowering=False, debug=False,
                   num_devices=N_CORES)
    xin = nc.dram_tensor("xin", [KI, P, KF], f16, kind="ExternalInput")
    w2p = nc.dram_tensor("w2p", [OC, P, HC * 512], f16, kind="ExternalInput")
    out_t = nc.dram_tensor("out_t", [OUT_F, C], f32, kind="ExternalOutput")

    relu = mybir.ActivationFunctionType.Relu

    with TileContext(nc) as tc:
        with (
            tc.tile_pool(name="const", bufs=1) as const,
            tc.tile_pool(name="psumA", bufs=3, space="PSUM") as psumA,
            tc.tile_pool(name="psumB", bufs=2, space="PSUM") as psumB,
            tc.tile_pool(name="psumW", bufs=1, space="PSUM") as psumW,
            tc.tile_pool(name="outp", bufs=4) as outp,
        ):
            # Warm tile on gpsimd (otherwise idle) so the sync/scalar rings
            # can start their DMA triggers immediately.
            wsrc = const.tile([P, 64], bf16, tag="warm")
            nc.gpsimd.memset(wsrc[:], 0.0)

            xin_ks = [const.tile([P, KF], f16, tag=f"xin_{k}",
                                 name=f"xin_{k}") for k in range(KI)]
            w2_cs = [const.tile([P, HC * 512], f16, tag=f"w2_{oc}",
                                name=f"w2_{oc}") for oc in range(OC)]
            # Sync ring: xin k0, then W2 c0/c2/c3.
            nc.sync.dma_start(xin_ks[0][:], xin[0])
            nc.sync.dma_start(w2_cs[0][:], w2p[0])
            nc.sync.dma_start(w2_cs[2][:], w2p[2])
            nc.sync.dma_start(w2_cs[3][:], w2p[3])
            # Scalar ring: xin k1, act-table preload, W2 c1.
            nc.scalar.dma_start(xin_ks[1][:], xin[1])
            wact = const.tile([P, 8], f32, tag="wact")
            nc.scalar.activation(wact[:], wsrc[:, :8], relu)
            nc.scalar.dma_start(w2_cs[1][:], w2p[1])

            b1_ap = xin_ks[0][:, C + HID: C + HID + HC]      # [P, 4] f16
            # tensor_scalar wants an f32 scalar operand; widen the f16 b2
            # tail once on the otherwise-idle gpsimd engine.
            b2_s = const.tile([P, OFT], f32, tag="b2s")
            nc.gpsimd.tensor_copy(b2_s[:], xin_ks[1][:, C + HID:
                                                     C + HID + OFT])
            b2_ap = b2_s

            # HAM warmup: the PE is promoted to full clock only after ~6us
            # of gap-free activity, so keep it busy from the earliest
            # possible moment until xin lands. Phase 1 uses the framework's
            # preamble-memset const AP (no dependency on our own memset);
            # phase 2 uses the bf16 warm tile.
            cone = nc.const_aps.aps[(bf16, 1.0)]
            wps = psumW.tile([64, 64], f32, tag="warmps")
            for _ in range(N_WARM0):
                nc.tensor.matmul(wps[:1, :1], lhsT=cone[:, :1],
                                 rhs=cone[:, :1], start=True, stop=True)
            for _ in range(N_WARM):
                nc.tensor.matmul(wps[:], lhsT=wsrc[:, :64], rhs=wsrc[:, :64],
                                 start=True, stop=True)

            # Stage A: H^T [hid(part), sample(free)], relu(x @ W1 + b1).
            ht = const.tile([P, HC, C], f16)
            for (s0, sn) in sgroups:
                for hc in range(HC):
                    ps = psumA.tile([P, 512], f32, tag="psA")
                    for k in range(KI):
                        nc.tensor.matmul(
                            ps[:, :sn],
                            lhsT=xin_ks[k][:, C + hc * P: C + (hc + 1) * P],
                            rhs=xin_ks[k][:, s0:s0 + sn],
                            start=(k == 0), stop=(k == KI - 1),
                        )
                    nc.scalar.activation(
                        ht[:, hc, s0:s0 + sn], ps[:, :sn], relu,
                        bias=b1_ap[:, hc:hc + 1],
                    )

            # Stage B: out^T[of-tile] = sum_hc W2[hc,of].T @ H^T[hc] + b2.
            # Samples are the moving dim, so cost tracks C exactly; the
            # per-partition b2 rides the DVE PSUM->SBUF move.
            ndma = 0
            for of in range(OFT):
                oc, o0 = of // (OFT // OC), (of % (OFT // OC)) * P
                pss = []
                for gi, (s0, sn) in enumerate(sgroups):
                    ps = psumB.tile([P, 512], f32, tag=f"psB{gi}",
                                    name=f"psB{gi}")
                    pss.append(ps)
                    for hc in range(HC):
                        nc.tensor.matmul(
                            ps[:, :sn],
                            lhsT=w2_cs[oc][:, hc * 512 + o0:
                                           hc * 512 + o0 + P],
                            rhs=ht[:, hc, s0:s0 + sn],
                            start=(hc == 0), stop=(hc == HC - 1),
                        )
                ot = outp.tile([P, C], f32, tag="ot")
                last = (of == OFT - 1)
                for gi, (s0, sn) in enumerate(sgroups):
                    nc.vector.tensor_scalar_add(
                        out=ot[:, s0:s0 + sn],
                        in0=pss[gi][:, :sn],
                        scalar1=b2_ap[:, of:of + 1],
                    )
                    if last:
                        eng = nc.scalar if ndma % 2 == 0 else nc.sync
                        eng.dma_start(out_t[of * P:(of + 1) * P, s0:s0 + sn],
                                      ot[:, s0:s0 + sn])
                        ndma += 1
                if not last:
                    eng = nc.scalar if ndma % 2 == 0 else nc.sync
                    eng.dma_start(out_t[of * P:(of + 1) * P, :], ot[:, :C])
                    ndma += 1

    nc.compile()
    return nc


def kernel(X, X_head_idx, W1, b1, W2, b2):
    X = np.ascontiguousarray(np.asarray(X, dtype=np.float32))
    idx = np.asarray(X_head_idx).astype(np.int64)
    W1 = np.asarray(W1, dtype=np.float32)
    b1 = np.asarray(b1, dtype=np.float32)
    W2 = np.asarray(W2, dtype=np.float32)
    b2 = np.asarray(b2, dtype=np.float32)

    batch = X.shape[0]
    counts = np.bincount(idx, minlength=N_HEADS)
    order = np.argsort(idx, kind="stable")
    positions = np.split(order, np.cumsum(counts)[:-1])

    C = max(512, int(-(-counts.max() // 16)) * 16)
    if C not in _NC_CACHE:
        _NC_CACHE[C] = build_nc(C)
    nc = _NC_CACHE[C]

    in_maps = []
    for h in range(N_HEADS):
        pos = positions[h]
        # xin[k, p, :] = [ X[pos, k*128+p] (C, padded) | W1[h, k*128+p, :]
        #                | bias tail (k0: b1, k1: b2 per of-tile) ]
        xin = np.zeros((KI, P, C + HID + 16), dtype=np.float16)
        if len(pos):
            xk = X[pos].T.reshape(KI, P, len(pos))          # [k, p, c]
            xin[:, :, :len(pos)] = xk
        xin[:, :, C:C + HID] = W1[h].reshape(KI, P, HID)
        xin[0, :, C + HID:C + HID + HC] = b1[h].reshape(HC, P).T
        xin[1, :, C + HID:C + HID + OFT] = b2[h].reshape(OFT, P).T
        # w2 packed: [oc, p, hc*512 + o'] = W2[h, hc*128 + p, oc*512 + o']
        w2t = np.transpose(W2[h].reshape(HC, P, OUT_F), (1, 0, 2))  # [p,hc,of]
        w2p = np.empty((OC, P, HC * 512), dtype=np.float16)
        for oc in range(OC):
            w2p[oc] = w2t[:, :, oc * 512:(oc + 1) * 512].reshape(P, HC * 512)
        in_maps.append({"xin": xin, "w2p": w2p})

    try:
        res = run_bass_kernel_spmd(nc, in_maps, list(range(N_CORES)))
    except Exception:
        res = run_bass_kernel_spmd(nc, in_maps, list(range(N_CORES)))

    out = np.empty((batch, OUT_F), dtype=np.float32)
    for h in range(N_HEADS):
        pos = positions[h]
        if len(pos):
            out[pos] = res.results[h]["out_t"][:, :len(pos)].T
    return out


# revision 19
# speedup vs baseline: 1.3247x; 1.0785x over previous
"""MultiHeadDecoder (moe_routing) Trainium2 kernel.

Strategy: expert-parallel. Each of the 8 cores owns one head's weights.
Host groups samples by head index, pads each group to a common capacity C
(multiple of 16, >= 512), and transposes X so the contraction dim lands on
partitions. Each core runs a dense 2-layer MLP (256->512 relu, 512->2048)
for its head's samples. The kernel returns out^T [OUT_F, C]; the host
transposes and scatters rows back to original order.

Layer 1 computes H^T (hid on partitions):
  H^T[hc] = W1[:, hc].T @ X^T          (lhsT=W1 chunk, rhs=X^T chunk)
Layer 2 keeps out-features on partitions and streams samples:
  out^T[of] = W2[:, of].T @ H^T        (lhsT=W2 [hid,of] tile, rhs=H^T)
so the per-matmul cost scales with the true sample count (split into two
~C/2 column groups to fit PSUM banks) instead of paying full 512-column
matmuls for a mostly-empty tail sample tile. The b2 bias is per-partition
in this layout, so a DVE tensor_scalar_add doubles as the PSUM->SBUF move.

All matmul operands are float16: full PE rate at any free size, FWL weight
loads, half the HBM traffic of f32, and lower PE power (less HAM duty
throttling). PSUM accumulates in f32.

Startup choreography (the original baseline lost ~7us here):
 - The PE is promoted to full clock only after ~6us of gap-free activity,
   so dummy matmuls run from the earliest possible moment (phase 1 reads a
   framework const AP that is memset during the preamble) until xin lands.
 - A dummy activation pulls ACT_TABLE_LOAD (~1.3us) off the critical path.
 - b1/b2 are folded into the xin tails: separate tiny-element DMAs have
   brutal per-descriptor cost and gated stage A by ~2us.
 - Inputs stream on both HWDGE rings in parallel (sync: xin k0 + W2
   c0/c2/c3; scalar: xin k1 + W2 c1). Output stores alternate rings, and
   the final of-tile's add+store is split to shorten the tail chain.
"""

import ml_dtypes
import numpy as np

import concourse.bass as bass
import concourse.mybir as mybir
from concourse import bacc
from concourse.tile import TileContext
from concourse.bass_utils import run_bass_kernel_spmd

IN_F, HID, OUT_F, N_HEADS, BATCH = 256, 512, 2048, 8, 4096
N_CORES = 8
P = 128
KI = IN_F // P     # 2  input-feature chunks
HC = HID // P      # 4  hidden chunks
OC = OUT_F // 512  # 4  W2 dma chunks of 512 out-features
OFT = OUT_F // P   # 16 out-feature tiles

f32 = mybir.dt.float32
f16 = mybir.dt.float16
bf16 = mybir.dt.bfloat16

N_WARM0 = 12   # tiny matmuls on the framework's const AP (no memset dep)
N_WARM = 52    # 64-col dummies bridging until xin lands (~10.6us)

_NC_CACHE: dict = {}


def _sgroups(C: int):
    """Split C sample columns into <=512-wide groups for PSUM banks."""
    if C <= 512:
        return [(0, C)]
    g0 = ((C // 2 + 15) // 16) * 16
    return [(0, g0), (g0, C - g0)]


def build_nc(C: int):
    """Build the per-core Bass program for capacity C (mult of 16, >=512)."""
    # xin[k] free layout: [ X^T (C) | W1 k-part (HID) | bias (16) ]
    # k=0 bias cols: b1 (HC=4 used); k=1 bias cols: b2 per of-tile (16).
    KF = C + HID + 16
    sgroups = _sgroups(C)

    nc = bacc.Bacc("TRN2", target_bir_lowering=False, debug=False,
                   num_devices=N_CORES)
    xin = nc.dram_tensor("xin", [KI, P, KF], f16, kind="ExternalInput")
    w2p = nc.dram_tensor("w2p", [OC, P, HC * 512], f16, kind="ExternalInput")
    out_t = nc.dram_tensor("out_t", [OUT_F, C], f32, kind="ExternalOutput")

    relu = mybir.ActivationFunctionType.Relu

    with TileContext(nc) as tc:
        with (
            tc.tile_pool(name="const", bufs=1) as const,
            tc.tile_pool(name="psumA", bufs=3, space="PSUM") as psumA,
            tc.tile_pool(name="psumB", bufs=2, space="PSUM") as psumB,
            tc.tile_pool(name="psumW", bufs=1, space="PSUM") as psumW,
            tc.tile_pool(name="outp", bufs=6) as outp,
        ):
            # Warm tile on gpsimd (otherwise idle) so the sync/scalar rings
            # can start their DMA triggers immediately.
            wsrc = const.tile([P, 64], bf16, tag="warm")
            nc.gpsimd.memset(wsrc[:], 0.0)

            xin_ks = [const.tile([P, KF], f16, tag=f"xin_{k}",
                                 name=f"xin_{k}") for k in range(KI)]
            w2_cs = [const.tile([P, HC * 512], f16, tag=f"w2_{oc}",
                                name=f"w2_{oc}") for oc in range(OC)]
            # Sync ring: xin k0, then W2 c0/c2 (balanced with scalar).
            nc.sync.dma_start(xin_ks[0][:], xin[0])
            nc.sync.dma_start(w2_cs[0][:], w2p[0])
            nc.sync.dma_start(w2_cs[2][:], w2p[2])
            # Scalar ring: xin k1, act-table preload, W2 c1/c3.
            nc.scalar.dma_start(xin_ks[1][:], xin[1])
            wact = const.tile([P, 8], f32, tag="wact")
            nc.scalar.activation(wact[:], wsrc[:, :8], relu)
            nc.scalar.dma_start(w2_cs[1][:], w2p[1])
            nc.scalar.dma_start(w2_cs[3][:], w2p[3])

            b1_ap = xin_ks[0][:, C + HID: C + HID + HC]      # [P, 4] f16
            # tensor_scalar wants an f32 scalar operand; widen the f16 b2
            # tail once on the otherwise-idle gpsimd engine.
            b2_s = const.tile([P, OFT], f32, tag="b2s")
            nc.gpsimd.tensor_copy(b2_s[:], xin_ks[1][:, C + HID:
                                                     C + HID + OFT])
            b2_ap = b2_s

            # HAM warmup: the PE is promoted to full clock only after ~6us
            # of gap-free activity, so keep it busy from the earliest
            # possible moment until xin lands. Phase 1 uses the framework's
            # preamble-memset const AP (no dependency on our own memset);
            # phase 2 uses the bf16 warm tile.
            cone = nc.const_aps.aps[(bf16, 1.0)]
            wps = psumW.tile([64, 64], f32, tag="warmps")
            for _ in range(N_WARM0):
                nc.tensor.matmul(wps[:1, :1], lhsT=cone[:, :1],
                                 rhs=cone[:, :1], start=True, stop=True)
            for _ in range(N_WARM):
                nc.tensor.matmul(wps[:], lhsT=wsrc[:, :64], rhs=wsrc[:, :64],
                                 start=True, stop=True)

            # Stage A: H^T [hid(part), sample(free)], relu(x @ W1 + b1).
            ht = const.tile([P, HC, C], f16)
            for (s0, sn) in sgroups:
                for hc in range(HC):
                    ps = psumA.tile([P, 512], f32, tag="psA")
                    for k in range(KI):
                        nc.tensor.matmul(
                            ps[:, :sn],
                            lhsT=xin_ks[k][:, C + hc * P: C + (hc + 1) * P],
                            rhs=xin_ks[k][:, s0:s0 + sn],
                            start=(k == 0), stop=(k == KI - 1),
                        )
                    nc.scalar.activation(
                        ht[:, hc, s0:s0 + sn], ps[:, :sn], relu,
                        bias=b1_ap[:, hc:hc + 1],
                    )

            # Stage B: out^T[of-tile] = sum_hc W2[hc,of].T @ H^T[hc] + b2.
            # Samples are the moving dim, so cost tracks C exactly; the
            # per-partition b2 rides the DVE PSUM->SBUF move.
            ndma = 0
            for of in range(OFT):
                oc, o0 = of // (OFT // OC), (of % (OFT // OC)) * P
                pss = []
                for gi, (s0, sn) in enumerate(sgroups):
                    ps = psumB.tile([P, 512], f32, tag=f"psB{gi}",
                                    name=f"psB{gi}")
                    pss.append(ps)
                    for hc in range(HC):
                        nc.tensor.matmul(
                            ps[:, :sn],
                            lhsT=w2_cs[oc][:, hc * 512 + o0:
                                           hc * 512 + o0 + P],
                            rhs=ht[:, hc, s0:s0 + sn],
                            start=(hc == 0), stop=(hc == HC - 1),
                        )
                ot = outp.tile([P, C], f32, tag="ot")
                if of < OFT - 1:
                    for gi, (s0, sn) in enumerate(sgroups):
                        nc.vector.tensor_scalar_add(
                            out=ot[:, s0:s0 + sn],
                            in0=pss[gi][:, :sn],
                            scalar1=b2_ap[:, of:of + 1],
                        )
                    eng = nc.scalar if ndma % 2 == 0 else nc.sync
                    eng.dma_start(out_t[of * P:(of + 1) * P, :], ot[:, :C])
                    ndma += 1
                else:
                    # Final of-tile: add+store in small chunks so the tail
                    # chain (add -> trigger -> transfer -> HBM ack) is as
                    # short as possible; the very last chunk is 128 cols.
                    chunks = []
                    for gi, (s0, sn) in enumerate(sgroups):
                        h = ((sn // 2 + 15) // 16) * 16
                        chunks.append((gi, s0, h))
                        if sn - h:
                            chunks.append((gi, s0 + h, sn - h))
                    for (gi, s0, sn) in chunks:
                        g0 = sgroups[gi][0]
                        nc.vector.tensor_scalar_add(
                            out=ot[:, s0:s0 + sn],
                            in0=pss[gi][:, s0 - g0:s0 - g0 + sn],
                            scalar1=b2_ap[:, of:of + 1],
                        )
                        eng = nc.scalar if ndma % 2 == 0 else nc.sync
                        eng.dma_start(out_t[of * P:(of + 1) * P, s0:s0 + sn],
                                      ot[:, s0:s0 + sn])
                        ndma += 1

    nc.compile()
    return nc


def kernel(X, X_head_idx, W1, b1, W2, b2):
    X = np.ascontiguousarray(np.asarray(X, dtype=np.float32))
    idx = np.asarray(X_head_idx).astype(np.int64)
    W1 = np.asarray(W1, dtype=np.float32)
    b1 = np.asarray(b1, dtype=np.float32)
    W2 = np.asarray(W2, dtype=np.float32)
    b2 = np.asarray(b2, dtype=np.float32)

    batch = X.shape[0]
    counts = np.bincount(idx, minlength=N_HEADS)
    order = np.argsort(idx, kind="stable")
    positions = np.split(order, np.cumsum(counts)[:-1])

    C = max(512, int(-(-counts.max() // 16)) * 16)
    if C not in _NC_CACHE:
        _NC_CACHE[C] = build_nc(C)
    nc = _NC_CACHE[C]

    in_maps = []
    for h in range(N_HEADS):
        pos = positions[h]
        # xin[k, p, :] = [ X[pos, k*128+p] (C, padded) | W1[h, k*128+p, :]
        #                | bias tail (k0: b1, k1: b2 per of-tile) ]
        xin = np.zeros((KI, P, C + HID + 16), dtype=np.float16)
        if len(pos):
            xk = X[pos].T.reshape(KI, P, len(pos))          # [k, p, c]
            xin[:, :, :len(pos)] = xk
        xin[:, :, C:C + HID] = W1[h].reshape(KI, P, HID)
        xin[0, :, C + HID:C + HID + HC] = b1[h].reshape(HC, P).T
        xin[1, :, C + HID:C + HID + OFT] = b2[h].reshape(OFT, P).T
        # w2 packed: [oc, p, hc*512 + o'] = W2[h, hc*128 + p, oc*512 + o']
        w2t = np.transpose(W2[h].reshape(HC, P, OUT_F), (1, 0, 2))  # [p,hc,of]
        w2p = np.empty((OC, P, HC * 512), dtype=np.float16)
        for oc in range(OC):
            w2p[oc] = w2t[:, :, oc * 512:(oc + 1) * 512].reshape(P, HC * 512)
        in_maps.append({"xin": xin, "w2p": w2p})

    try:
        res = run_bass_kernel_spmd(nc, in_maps, list(range(N_CORES)))
    except Exception:
        res = run_bass_kernel_spmd(nc, in_maps, list(range(N_CORES)))

    out = np.empty((batch, OUT_F), dtype=np.float32)
    for h in range(N_HEADS):
        pos = positions[h]
        if len(pos):
            out[pos] = res.results[h]["out_t"][:, :len(pos)].T
    return out
